# revision 13
# baseline (speedup 1.0000x reference)
"""Trainium2 Bass kernel for BasicEdgeModel (edge-wise MLP with node gathers).

y[e] = relu(concat(x[src_e], x[tgt_e], edge_attr[e]) @ W1 + b1) @ W2 + b2

Strategy (8 NeuronCores, data-parallel over edges):
  - Two bf16 node tables replicated per core: tabA = [x | 0], tabB = [0 | x]
    (rows padded to 128 cols = 256B so dma_gather(transpose=True) works).
  - dma_gather(transpose=True) fetches node rows as COLUMNS: gA[f, j] =
    tabA[srcA_j, f] -> features on partitions, edges on the free dim. No PE
    transposes anywhere.
  - int16 gather indices: nodes bucketed by 32768; edges sorted on host into
    16 (src_bucket, tgt_bucket) groups laid out on a fixed-capacity grid.
  - Per 512-edge block: PSUM = W1AB.T@gA + W1AB.T@gB + W1C.T@eaT; ACT does
    bias+relu into bf16 hT; W2.T@hT -> [64, 512] blocks; pairs of blocks are
    stacked on partitions into a packed [128, E_GRID/2] f32 output for
    full-width stores. Host decodes + unpermutes.
"""

import numpy as np
import ml_dtypes

import concourse.bass as bass
import concourse.mybir as mybir
import concourse.tile as tile
from concourse import bacc
from concourse.bass_utils import run_bass_kernel_spmd

# problem geometry (fixed by the task)
N_NODES = 100000
NODE_DIM = 64
EDGE_DIM = 32
HIDDEN = 128
OUT_DIM = 64
N_EDGES = 1600000
N_CORES = 8
E_CORE = N_EDGES // N_CORES   # 200000

BUCKET = 32768                # int16 index range per table slice
N_BUCKET = 4                  # ceil(100000 / 32768)
N_GROUP = N_BUCKET * N_BUCKET # 16 (src_bucket, tgt_bucket) groups


def _group_caps(e_core=E_CORE, n_nodes=N_NODES):
    """Per-group slot capacities: mean + >7 sigma, rounded to 512."""
    p = np.minimum(
        np.arange(1, N_BUCKET + 1) * BUCKET, n_nodes
    ) - np.arange(N_BUCKET) * BUCKET
    p = p / n_nodes  # bucket probabilities
    caps = []
    for bs in range(N_BUCKET):
        for bt in range(N_BUCKET):
            mean = e_core * p[bs] * p[bt]
            sig = np.sqrt(max(mean, 1.0))
            cap = int(np.ceil((mean + 8 * sig + 256) / 512) * 512)
            caps.append(max(cap, 512))
    # make total block count even so 512-block pairs fill the packed output
    if (sum(caps) // 512) % 2:
        caps[-1] += 512
    return caps


def _segments(cap, seg_max):
    """Split cap into gather segments, each %128 and <= seg_max."""
    segs = []
    rem = cap
    while rem > 0:
        s = min(rem, seg_max)
        segs.append(s)
        rem -= s
    assert all(x % 128 == 0 for x in segs)
    return segs


GROUP_CAPS = _group_caps()
SEG_MAX = 5632
E_GRID = sum(GROUP_CAPS)

BF16 = mybir.dt.bfloat16
F32 = mybir.dt.float32
I16 = mybir.dt.int16
AF = mybir.ActivationFunctionType

TRACE = False
TRACE_TMPDIR = None
LAST_RESULT = None


def build_nc(n_nodes, caps, seg_max):
    e_grid = sum(caps)
    assert e_grid % 1024 == 0
    # flat segment table: (group, slot_base, seg_len, idx_col_base)
    segtab = []
    icols = 0
    for g in range(N_GROUP):
        base = sum(caps[:g])
        off = 0
        for L in _segments(caps[g], seg_max):
            segtab.append((g, base + off, L, icols))
            icols += L // 16
            off += L

    nc = bacc.Bacc()
    tabA = nc.declare_dram_parameter("tabA", [n_nodes, 128], BF16, isOutput=False)
    tabB = nc.declare_dram_parameter("tabB", [n_nodes, 128], BF16, isOutput=False)
    idxA = nc.declare_dram_parameter("idxA", [128, icols], I16, isOutput=False)
    idxB = nc.declare_dram_parameter("idxB", [128, icols], I16, isOutput=False)
    eat = nc.declare_dram_parameter("eat", [EDGE_DIM, e_grid], BF16, isOutput=False)
    w1ab = nc.declare_dram_parameter("w1ab", [2 * NODE_DIM, HIDDEN], BF16, isOutput=False)
    w1c = nc.declare_dram_parameter("w1c", [EDGE_DIM, HIDDEN], BF16, isOutput=False)
    w2 = nc.declare_dram_parameter("w2", [HIDDEN, OUT_DIM], BF16, isOutput=False)
    b1 = nc.declare_dram_parameter("b1", [HIDDEN, 1], F32, isOutput=False)
    b2 = nc.declare_dram_parameter("b2", [OUT_DIM, 1], F32, isOutput=False)
    out = nc.declare_dram_parameter("out", [128, e_grid // 2], F32, isOutput=True)

    with tile.TileContext(nc) as tc:
        with (
            tc.tile_pool(name="const", bufs=1) as cp,
            tc.tile_pool(name="idxp", bufs=3) as idxp,
            tc.tile_pool(name="gap", bufs=3) as gap,
            tc.tile_pool(name="eap", bufs=4) as eap,
            tc.tile_pool(name="htp", bufs=4) as htp,
            tc.tile_pool(name="osp", bufs=4) as osp,
            tc.tile_pool(name="hps", bufs=4, space="PSUM") as hps,
            tc.tile_pool(name="ops", bufs=4, space="PSUM") as ops,
        ):
            w1ab_t = cp.tile([2 * NODE_DIM, HIDDEN], BF16)
            nc.sync.dma_start(out=w1ab_t[:], in_=w1ab[:])
            w1c_t = cp.tile([EDGE_DIM, HIDDEN], BF16)
            nc.sync.dma_start(out=w1c_t[:], in_=w1c[:])
            w2_t = cp.tile([HIDDEN, OUT_DIM], BF16)
            nc.sync.dma_start(out=w2_t[:], in_=w2[:])
            b1_t = cp.tile([HIDDEN, 1], F32)
            nc.sync.dma_start(out=b1_t[:], in_=b1[:])
            b2_t = cp.tile([OUT_DIM, 1], F32)
            nc.sync.dma_start(out=b2_t[:], in_=b2[:])

            for (g, slot_base, seg_len, icol) in segtab:
                baseA = (g // N_BUCKET) * BUCKET
                baseB = (g % N_BUCKET) * BUCKET
                nw = seg_len // 16
                ia_t = idxp.tile([128, nw], I16, tag="ia", padded_shape=[128, seg_max // 16])
                nc.sync.dma_start(out=ia_t[:], in_=idxA[:, icol:icol + nw])
                ib_t = idxp.tile([128, nw], I16, tag="ib", padded_shape=[128, seg_max // 16])
                nc.sync.dma_start(out=ib_t[:], in_=idxB[:, icol:icol + nw])

                gA = gap.tile([128, 1, seg_len], BF16, tag="ga",
                              padded_shape=[128, 1, seg_max])
                nc.gpsimd.dma_gather(
                    gA[:], tabA[baseA:, :], ia_t[:], seg_len, seg_len,
                    128, transpose=True, single_packet=False,
                )
                gB = gap.tile([128, 1, seg_len], BF16, tag="gb",
                              padded_shape=[128, 1, seg_max])
                nc.gpsimd.dma_gather(
                    gB[:], tabB[baseB:, :], ib_t[:], seg_len, seg_len,
                    128, transpose=True, single_packet=False,
                )
                ea_t = eap.tile([EDGE_DIM, seg_len], BF16,
                                padded_shape=[EDGE_DIM, seg_max])
                nc.sync.dma_start(
                    out=ea_t[:], in_=eat[:, slot_base:slot_base + seg_len]
                )

                for b in range(seg_len // 512):
                    blk = slot_base // 512 + b
                    sl = slice(b * 512, (b + 1) * 512)
                    hp = hps.tile([128, 512], F32, space="PSUM")
                    nc.tensor.matmul(
                        hp[:], lhsT=w1ab_t[:], rhs=gA[:, 0, sl],
                        start=True, stop=False,
                    )
                    nc.tensor.matmul(
                        hp[:], lhsT=w1ab_t[:], rhs=gB[:, 0, sl],
                        start=False, stop=False,
                    )
                    nc.tensor.matmul(
                        hp[:], lhsT=w1c_t[:], rhs=ea_t[:, sl],
                        start=False, stop=True,
                    )
                    hT = htp.tile([128, 512], BF16)
                    nc.scalar.activation(
                        out=hT[:], in_=hp[:], func=AF.Relu,
                        bias=b1_t[:, :1], scale=1.0,
                    )
                    op = ops.tile([OUT_DIM, 512], F32, space="PSUM")
                    nc.tensor.matmul(
                        op[:], lhsT=w2_t[:], rhs=hT[:], start=True, stop=True,
                    )
                    j = blk % 2
                    if j == 0:
                        st = osp.tile([128, 512], F32)
                    nc.vector.tensor_tensor(
                        out=st[j * OUT_DIM:(j + 1) * OUT_DIM, :],
                        in0=op[:],
                        in1=b2_t[:, :1].to_broadcast([OUT_DIM, 512]),
                        op=mybir.AluOpType.add,
                    )
                    if j == 1:
                        col = (blk // 2) * 512
                        nc.sync.dma_start(out=out[:, col:col + 512], in_=st[:])

    nc.compile()
    return nc


def _wrap_idx(v):
    """[n] int -> [128, n/16] int16 (idx j at [j%16, j//16]), replicated x8."""
    n = v.shape[0]
    w = v.reshape(n // 16, 16).T.astype(np.int16)
    return np.tile(w, (8, 1))


def _prep_core(src, tgt, ea, n_nodes, caps, seg_max):
    """Sort this core's edges into the (src_bucket, tgt_bucket) grid.

    Returns idxA, idxB ([128, icols] int16), eaT ([32, e_grid] bf16) and
    slot_of_edge ([n] int64) mapping original edge -> grid slot."""
    e_grid = sum(caps)
    n = src.shape[0]

    grp = (src >> 15) * N_BUCKET + (tgt >> 15)
    order = np.argsort(grp, kind="stable")
    counts = np.bincount(grp, minlength=N_GROUP)
    if np.any(counts > np.asarray(caps)):
        raise RuntimeError(f"group overflow: {counts} vs {caps}")

    bases = np.concatenate([[0], np.cumsum(caps)[:-1]])
    slot_of_sorted = np.empty(n, np.int64)
    start = 0
    for g in range(N_GROUP):
        c = counts[g]
        slot_of_sorted[start:start + c] = bases[g] + np.arange(c)
        start += c
    slot_of_edge = np.empty(n, np.int64)
    slot_of_edge[order] = slot_of_sorted

    srcs = np.zeros(e_grid, np.int32)
    tgts = np.zeros(e_grid, np.int32)
    # padding slots gather relative row 0 of their bucket (always valid)
    for g in range(N_GROUP):
        srcs[bases[g]:bases[g] + caps[g]] = (g // N_BUCKET) * BUCKET
        tgts[bases[g]:bases[g] + caps[g]] = (g % N_BUCKET) * BUCKET
    srcs[slot_of_edge] = src
    tgts[slot_of_edge] = tgt

    wrapsA, wrapsB = [], []
    for g in range(N_GROUP):
        off = 0
        for L in _segments(caps[g], seg_max):
            lo = bases[g] + off
            wrapsA.append(_wrap_idx(srcs[lo:lo + L] - (g // N_BUCKET) * BUCKET))
            wrapsB.append(_wrap_idx(tgts[lo:lo + L] - (g % N_BUCKET) * BUCKET))
            off += L
    idxA = np.concatenate(wrapsA, axis=1)
    idxB = np.concatenate(wrapsB, axis=1)

    eaT = np.zeros((EDGE_DIM, e_grid), ml_dtypes.bfloat16)
    eaT[:, slot_of_edge] = ea.T.astype(ml_dtypes.bfloat16)
    return idxA, idxB, eaT, slot_of_edge


def _decode_out(o, e_grid):
    """[128, e_grid//2] packed -> [e_grid, 64] in slot order."""
    O = o.reshape(2, OUT_DIM, e_grid // 1024, 512)  # (j, f, t, q)
    return O.transpose(2, 0, 3, 1).reshape(e_grid, OUT_DIM)


_NC_CACHE = {}


def kernel(x, edge_attr, W1, b1, W2, b2, edge_index):
    global LAST_RESULT
    x = np.asarray(x, np.float32)
    edge_attr = np.asarray(edge_attr, np.float32)
    W1 = np.asarray(W1, np.float32)
    b1 = np.asarray(b1, np.float32)
    W2 = np.asarray(W2, np.float32)
    b2 = np.asarray(b2, np.float32)
    edge_index = np.asarray(edge_index)

    key = "full"
    if key not in _NC_CACHE:
        _NC_CACHE[key] = build_nc(N_NODES, GROUP_CAPS, SEG_MAX)
    nc = _NC_CACHE[key]

    xbf = x.astype(ml_dtypes.bfloat16)
    tabA = np.zeros((N_NODES, 128), ml_dtypes.bfloat16)
    tabA[:, :NODE_DIM] = xbf
    tabB = np.zeros((N_NODES, 128), ml_dtypes.bfloat16)
    tabB[:, NODE_DIM:] = xbf

    w1ab = W1[:2 * NODE_DIM].astype(ml_dtypes.bfloat16)
    w1c = W1[2 * NODE_DIM:].astype(ml_dtypes.bfloat16)
    w2 = W2.astype(ml_dtypes.bfloat16)
    b1c = np.ascontiguousarray(b1.reshape(HIDDEN, 1))
    b2c = np.ascontiguousarray(b2.reshape(OUT_DIM, 1))

    src_all = edge_index[0].astype(np.int32)
    tgt_all = edge_index[1].astype(np.int32)

    in_maps = []
    slots = []
    for i in range(N_CORES):
        s, e = i * E_CORE, (i + 1) * E_CORE
        idxA, idxB, eaT, slot = _prep_core(
            src_all[s:e], tgt_all[s:e], edge_attr[s:e],
            N_NODES, GROUP_CAPS, SEG_MAX,
        )
        slots.append(slot)
        in_maps.append({
            "tabA": tabA, "tabB": tabB, "idxA": idxA, "idxB": idxB,
            "eat": eaT, "w1ab": w1ab, "w1c": w1c, "w2": w2,
            "b1": b1c, "b2": b2c,
        })

    res = run_bass_kernel_spmd(
        nc, in_maps, core_ids=list(range(N_CORES)), trace=TRACE,
        tmpdir=TRACE_TMPDIR,
    )
    LAST_RESULT = res
    outs = []
    for i in range(N_CORES):
        y_slots = _decode_out(np.asarray(res.results[i]["out"]), E_GRID)
        outs.append(y_slots[slots[i]])
    return np.ascontiguousarray(np.concatenate(outs, axis=0), dtype=np.float32)
